# revision 23
# baseline (speedup 1.0000x reference)
"""Data-parallel 3x3 conv (implicit GEMM) for Trainium2, 8 NeuronCores.

Problem: x (32,128,56,56) f32, W (256,1152) f32 [C_out, C_in*KH*KW, taps in
(ci,kh,kw) order], b (256,), stride 1, pad 1 -> out (32,256,56,56) f32.

Strategy
- Shard the batch dim across the 8 cores (4 images each); replicate W and b.
- Host-side prep: zero-pad x to 58x58 and cast x and W to bf16 (PE runs
  bf16 at the same 1 cycle/row as f32r, input rounding adds ~1.5e-3 rel
  err, far under the 2e-2 gate). W is pre-transposed to the stationary
  [ci, (co_tile, tap, co)] layout so the first co-half streams in first.
- All loads are plain HWDGE DMAs (no SWDGE descriptor generation, no
  casting): weights + bias on the sync (SP) ring, x on the vector (DVE)
  ring, output stores on the scalar (Act) ring. The first W/x chunks are
  sized so the 9-tap matmul stream can start ~3.4us in.
- Per core: keep the whole padded bf16 shard (3.4 MB) + weights in SBUF.
  For each (image, co-half, 8-row block): accumulate 9 shifted matmuls
  (one per tap) into one PSUM bank, contraction dim = C_in = 128, moving
  dim N = 8*56 = 448. ScalarE fuses the bias-add with the PSUM->SBUF
  copy; the result DMAs out from ScalarE's HWDGE ring.
- A short stream of dummy bf16 matmuls warms the PE p-state clock-gate
  during the initial DMA window, and the final 8-row block is split 6+2
  so the very last activation+store+drain chain is short.
"""

import numpy as np
import ml_dtypes

import bass_rust as _br
import concourse.bass as bass
import concourse.mybir as mybir
import concourse.tile as tile
from concourse.bass_utils import run_bass_kernel_spmd

N_CORES = 8
B, C_IN, H, W_ = 32, 128, 56, 56
C_OUT = 256
B_LOC = B // N_CORES          # 4 images per core
HP, WP = H + 2, W_ + 2        # padded 58x58
IMG_PAD = HP * WP             # 3364
ROWS_PER_BLK = 8              # 8 rows * 56 cols = 448 = moving dim
N_BLK = H // ROWS_PER_BLK     # 7
N_MOV = ROWS_PER_BLK * W_     # 448
N_TAPS = 9
CO_TILES = C_OUT // 128       # 2

_F32 = mybir.dt.float32
_BF16 = mybir.dt.bfloat16

# x image-0 row chunks (DMA'd individually so the PE can start early);
# remaining images load in halves.
X0_ROW_CHUNKS = [(0, 10), (10, 22), (22, 34), (34, 46), (46, 58)]
XN_ROW_CHUNKS = [(0, 29), (29, 58)]
# W columns in [ci, (co_tile, tap, co)] layout: first co-half split so the
# first tap group streams just ahead of the matmuls.
W_COLS = N_TAPS * 128         # 1152 per co-half
W_CHUNKS = [(0, 640), (640, 1152)]   # co-half 0 (taps 0-4, taps 5-8)
N_WARMUP = 28                 # small dummy matmuls to ramp the PE clock


def _split_multi_waits(nc):
    """This walrus build accepts at most ONE sync-wait per instruction.

    Tile can emit several (e.g. a matmul waiting on two input DMAs, or the
    exit drain waiting on every outstanding semaphore). Hoist the extras onto
    injected same-engine NOPs immediately ahead of the offender — sequencers
    execute their stream in order, so the waits still all happen before it.
    """
    for bb in nc.m.functions[0].blocks:
        il = bb.instructions
        i = 0
        while i < len(il):
            inst = il[i]
            si = inst.sync_info
            w = list(si.on_wait) if (si and si.on_wait) else []
            if len(w) > 1:
                si.on_wait = w[-1:]
                for wi in w[:-1]:
                    nop = mybir.InstNoOp(
                        name=nc.get_next_instruction_name(), ins=[], outs=[]
                    )
                    nop.engine = inst.engine
                    nop.sync_info = _br.SyncInfo(on_wait=[wi], on_update=[])
                    nc.register_instruction(nop)
                    il.insert(i, nop)
                    i += 1
            i += 1


def _build_program():
    nc = bass.Bass("TRN2", target_bir_lowering=False, debug=False,
                   num_devices=N_CORES)
    xp = nc.dram_tensor("xp", [B_LOC, C_IN, IMG_PAD], _BF16,
                        kind="ExternalInput").ap()
    wt = nc.dram_tensor("wt", [C_IN, CO_TILES * W_COLS], _BF16,
                        kind="ExternalInput").ap()
    bt = nc.dram_tensor("bt", [128, CO_TILES], _F32, kind="ExternalInput").ap()
    out = nc.dram_tensor("out", [B_LOC, C_OUT, H, W_], _F32,
                         kind="ExternalOutput").ap()

    with tile.TileContext(nc) as tc:
        with (
            tc.tile_pool(name="xpool", bufs=1) as xpool,
            tc.tile_pool(name="wpool", bufs=1) as wpool,
            tc.tile_pool(name="opool", bufs=4) as opool,
            tc.tile_pool(name="otail", bufs=3) as otail,
            tc.tile_pool(name="wmpool", bufs=1, space="PSUM") as wmpool,
            tc.tile_pool(name="ppool", bufs=4, space="PSUM") as ppool,
        ):
            # PE warm-up: the HAM clock-gate runs the PE at reduced speed
            # until it sees ~3us of sustained activity. The real stream can't
            # start until the first weight/activation chunks land (~3.4us),
            # so burn that window on dummy matmuls over a zeroed scratch tile
            # into a never-read PSUM bank.
            scratch = wpool.tile([128, 128], _BF16, tag="scratch")
            nc.gpsimd.memset(scratch[:], 0.0)
            wps = wmpool.tile([128, 128], _F32, tag="wps")
            for _ in range(N_WARMUP):
                nc.tensor.matmul(wps[:], scratch[:], scratch[:],
                                 start=True, stop=True)

            # Only SP (nc.sync) and Activation (nc.scalar) have HWDGE rings.
            # Startup-critical chunks alternate between them so ring prep
            # overlaps; the bulk image loads go on SP (Act issues the 57
            # activation+store pairs later).
            w_sb = wpool.tile([C_IN, CO_TILES * W_COLS], _BF16, tag="w")
            b_sb = wpool.tile([128, CO_TILES], _F32, tag="b")
            x_sb = []
            for n in range(B_LOC):
                xt = xpool.tile([C_IN, IMG_PAD], _BF16, tag=f"x{n}",
                                name=f"x{n}")
                x_sb.append(xt)

            def xdma(eng, n, r0, r1):
                lo, hi = r0 * WP, r1 * WP
                eng.dma_start(x_sb[n][:, lo:hi], xp[n][:, lo:hi])

            nc.sync.dma_start(w_sb[:, 0:640], wt[:, 0:640])        # taps 0-4
            xdma(nc.scalar, 0, 0, 10)
            nc.sync.dma_start(w_sb[:, 640:1152], wt[:, 640:1152])  # taps 5-8
            xdma(nc.scalar, 0, 10, 22)
            nc.sync.dma_start(b_sb[:], bt[:])
            xdma(nc.sync, 0, 22, 34)
            xdma(nc.scalar, 0, 34, 46)
            xdma(nc.sync, 0, 46, 58)
            nc.scalar.dma_start(w_sb[:, W_COLS:], wt[:, W_COLS:])  # co-half 1
            for n in range(1, B_LOC):
                for r0, r1 in XN_ROW_CHUNKS:
                    xdma(nc.sync, n, r0, r1)

            def do_group(n, t, r0, nrows, store_eng, opool=opool,
                         act_on_dve=False):
                """One PSUM accumulation group: rows [r0, r0+nrows) of the
                output for image n, co-half t."""
                nmov = nrows * W_
                xv = x_sb[n][:].rearrange("p (h w) -> p h w", h=HP, w=WP)
                ps = ppool.tile([128, nmov], _F32, tag="ps")
                for k in range(N_TAPS):
                    kh, kw = divmod(k, 3)
                    rhs = xv[:, r0 + kh:r0 + kh + nrows, kw:kw + W_]
                    c0 = t * W_COLS + k * 128
                    nc.tensor.matmul(
                        ps[:], w_sb[:, c0:c0 + 128], rhs,
                        start=(k == 0),
                        stop=(k == N_TAPS - 1),
                    )
                o_sb = opool.tile([128, nmov], _F32, tag="o")
                if act_on_dve:
                    # Idle DVE does the PSUM->SBUF bias-add so the closing
                    # chain never waits on the Activation engine/SEQ.
                    nc.vector.tensor_scalar_add(o_sb[:], ps[:],
                                                b_sb[:, t:t + 1])
                else:
                    nc.scalar.activation(
                        o_sb[:], ps[:],
                        mybir.ActivationFunctionType.Identity,
                        bias=b_sb[:, t:t + 1],
                    )
                store_eng.dma_start(
                    out[n, bass.ts(t, 128), r0:r0 + nrows, :],
                    o_sb[:],
                )

            for n in range(B_LOC):
                for t in range(CO_TILES):
                    first_nt = (n == 0 and t == 0)
                    last_nt = (n == B_LOC - 1 and t == CO_TILES - 1)
                    if first_nt:
                        # The first 1-2 matmul instructions are costed at the
                        # mid p-state (they are dispatched before the warmup
                        # ramp completes) -- make them tiny so that penalty
                        # is small.
                        do_group(n, t, 0, 4, nc.scalar, opool=otail)
                        do_group(n, t, 4, 4, nc.scalar, opool=otail)
                        for j in range(1, N_BLK):
                            do_group(n, t, j * ROWS_PER_BLK, ROWS_PER_BLK,
                                     nc.scalar)
                    elif not last_nt:
                        for j in range(N_BLK):
                            do_group(n, t, j * ROWS_PER_BLK, ROWS_PER_BLK,
                                     nc.scalar)
                    else:
                        # Final (image, co-half): taper the block sizes so
                        # every act/store chain clears before the closer --
                        # j0..j4 full-size, two 6-row groups, then a 4-row
                        # closer whose bias-copy runs on the idle DVE and
                        # whose store goes out on the idle SP ring.
                        for j in range(5):
                            do_group(n, t, j * ROWS_PER_BLK, ROWS_PER_BLK,
                                     nc.scalar)
                        do_group(n, t, 40, 6, nc.sync, opool=otail)
                        do_group(n, t, 46, 6, nc.scalar, opool=otail)
                        do_group(n, t, 52, 4, nc.sync, opool=otail,
                                 act_on_dve=True)

    _split_multi_waits(nc)
    return nc


_CACHED_NC = None


def _get_program():
    global _CACHED_NC
    if _CACHED_NC is None:
        _CACHED_NC = _build_program()
    return _CACHED_NC


def _prep_inputs(x, W, b):
    xp_all = np.pad(x, ((0, 0), (0, 0), (1, 1), (1, 1))).astype(
        ml_dtypes.bfloat16
    )
    # [C_out, C_in*9] -> [ci, (co_tile, tap, co)]
    wt = np.ascontiguousarray(
        W.reshape(CO_TILES, 128, C_IN, N_TAPS)
        .transpose(2, 0, 3, 1)
        .reshape(C_IN, -1)
    ).astype(ml_dtypes.bfloat16)
    bt = np.ascontiguousarray(b.reshape(CO_TILES, 128).T)
    in_maps = []
    for i in range(N_CORES):
        shard = np.ascontiguousarray(
            xp_all[i * B_LOC:(i + 1) * B_LOC].reshape(B_LOC, C_IN, IMG_PAD)
        )
        in_maps.append({"xp": shard, "wt": wt, "bt": bt})
    return in_maps


def kernel(x, W, b):
    x = np.asarray(x, dtype=np.float32)
    W = np.asarray(W, dtype=np.float32)
    b = np.asarray(b, dtype=np.float32)
    nc = _get_program()
    in_maps = _prep_inputs(x, W, b)
    res = run_bass_kernel_spmd(nc, in_maps, list(range(N_CORES)), trace=False)
    return np.concatenate([res.results[i]["out"] for i in range(N_CORES)], axis=0)


# revision 28
# speedup vs baseline: 1.0022x; 1.0022x over previous
"""Data-parallel 3x3 conv (implicit GEMM) for Trainium2, 8 NeuronCores.

Problem: x (32,128,56,56) f32, W (256,1152) f32 [C_out, C_in*KH*KW, taps in
(ci,kh,kw) order], b (256,), stride 1, pad 1 -> out (32,256,56,56) f32.

Strategy (cost-model span ~103.0us/core vs 94.25us pure-matmul floor)
- Shard the batch dim across the 8 cores (4 images each); replicate W and b.
- Host-side prep: zero-pad x to 58x58 and cast x and W to bf16. The PE runs
  bf16 at the same 1 cycle/row as f32r (full rate), input rounding adds
  ~2e-3 rel err (gate is 2e-2), input DMA bytes halve, and unlike f32r
  there is no N>=256 moving-dim constraint, which the small head/tail
  groups below rely on. W is pre-transposed to the stationary
  [ci, (co_tile, tap, co)] layout so the first co-half streams in first.
- All loads are plain HWDGE DMAs (no SWDGE descriptor generation, no
  casting), interleaved over the two HWDGE rings (sync/SP and scalar/Act)
  so ring prep pipelines; chunk order is tuned so the first W taps and the
  first x rows land just as the PE p-state warm-up completes (~4.4us) and
  every later chunk arrives before its first consumer group.
- Per core: keep the whole padded bf16 shard (3.4 MB) + weights in SBUF.
  For each (image, co-half, row block): accumulate 9 shifted matmuls (one
  per 3x3 tap) into one PSUM bank, contraction dim = C_in = 128, moving
  dim N = rows*56 (448 for the 8-row steady-state blocks). ScalarE fuses
  the bias-add with the PSUM->SBUF copy and stores from its HWDGE ring.
  The matmul stream is gap-free: 504 tap-matmuls back-to-back.
- Head/tail shaping: the first block is split 4+4 so the two matmuls that
  get costed at the mid p-state are small; the final block tapers
  6+6+2+2, where the two closing 2-row groups are bias-copied by the
  otherwise idle DVE into one shared SBUF tile flushed by a single store
  on the idle SP ring, so the closing copy+store+sem+drain chain after
  the last matmul is ~3.9us instead of ~4.7us.
"""

import numpy as np
import ml_dtypes

import bass_rust as _br
import concourse.bass as bass
import concourse.mybir as mybir
import concourse.tile as tile
from concourse.bass_utils import run_bass_kernel_spmd

N_CORES = 8
B, C_IN, H, W_ = 32, 128, 56, 56
C_OUT = 256
B_LOC = B // N_CORES          # 4 images per core
HP, WP = H + 2, W_ + 2        # padded 58x58
IMG_PAD = HP * WP             # 3364
ROWS_PER_BLK = 8              # 8 rows * 56 cols = 448 = moving dim
N_BLK = H // ROWS_PER_BLK     # 7
N_MOV = ROWS_PER_BLK * W_     # 448
N_TAPS = 9
CO_TILES = C_OUT // 128       # 2

_F32 = mybir.dt.float32
_BF16 = mybir.dt.bfloat16

# x image-0 row chunks (DMA'd individually so the PE can start early);
# remaining images load in halves.
X0_ROW_CHUNKS = [(0, 10), (10, 22), (22, 34), (34, 46), (46, 58)]
XN_ROW_CHUNKS = [(0, 29), (29, 58)]
# W columns in [ci, (co_tile, tap, co)] layout: first co-half split so the
# first tap group streams just ahead of the matmuls.
W_COLS = N_TAPS * 128         # 1152 per co-half
W_CHUNKS = [(0, 640), (640, 1152)]   # co-half 0 (taps 0-4, taps 5-8)
N_WARMUP = 28                 # small dummy matmuls to ramp the PE clock


def _split_multi_waits(nc):
    """This walrus build accepts at most ONE sync-wait per instruction.

    Tile can emit several (e.g. a matmul waiting on two input DMAs, or the
    exit drain waiting on every outstanding semaphore). Hoist the extras onto
    injected same-engine NOPs immediately ahead of the offender — sequencers
    execute their stream in order, so the waits still all happen before it.
    """
    for bb in nc.m.functions[0].blocks:
        il = bb.instructions
        i = 0
        while i < len(il):
            inst = il[i]
            si = inst.sync_info
            w = list(si.on_wait) if (si and si.on_wait) else []
            if len(w) > 1:
                si.on_wait = w[-1:]
                for wi in w[:-1]:
                    nop = mybir.InstNoOp(
                        name=nc.get_next_instruction_name(), ins=[], outs=[]
                    )
                    nop.engine = inst.engine
                    nop.sync_info = _br.SyncInfo(on_wait=[wi], on_update=[])
                    nc.register_instruction(nop)
                    il.insert(i, nop)
                    i += 1
            i += 1


def _build_program():
    nc = bass.Bass("TRN2", target_bir_lowering=False, debug=False,
                   num_devices=N_CORES)
    xp = nc.dram_tensor("xp", [B_LOC, C_IN, IMG_PAD], _BF16,
                        kind="ExternalInput").ap()
    wt = nc.dram_tensor("wt", [C_IN, CO_TILES * W_COLS], _BF16,
                        kind="ExternalInput").ap()
    bt = nc.dram_tensor("bt", [128, CO_TILES], _F32, kind="ExternalInput").ap()
    out = nc.dram_tensor("out", [B_LOC, C_OUT, H, W_], _F32,
                         kind="ExternalOutput").ap()

    with tile.TileContext(nc) as tc:
        with (
            tc.tile_pool(name="xpool", bufs=1) as xpool,
            tc.tile_pool(name="wpool", bufs=1) as wpool,
            tc.tile_pool(name="opool", bufs=4) as opool,
            tc.tile_pool(name="otail", bufs=3) as otail,
            tc.tile_pool(name="wmpool", bufs=1, space="PSUM") as wmpool,
            tc.tile_pool(name="ppool", bufs=4, space="PSUM") as ppool,
        ):
            # PE warm-up: the HAM clock-gate runs the PE at reduced speed
            # until it sees ~3us of sustained activity. The real stream can't
            # start until the first weight/activation chunks land (~3.4us),
            # so burn that window on dummy matmuls over a zeroed scratch tile
            # into a never-read PSUM bank.
            scratch = wpool.tile([128, 128], _BF16, tag="scratch")
            nc.gpsimd.memset(scratch[:], 0.0)
            wps = wmpool.tile([128, 128], _F32, tag="wps")
            for _ in range(N_WARMUP):
                nc.tensor.matmul(wps[:], scratch[:], scratch[:],
                                 start=True, stop=True)

            # Only SP (nc.sync) and Activation (nc.scalar) have HWDGE rings.
            # Startup-critical chunks alternate between them so ring prep
            # overlaps; the bulk image loads go on SP (Act issues the 57
            # activation+store pairs later).
            w_sb = wpool.tile([C_IN, CO_TILES * W_COLS], _BF16, tag="w")
            b_sb = wpool.tile([128, CO_TILES], _F32, tag="b")
            x_sb = []
            for n in range(B_LOC):
                xt = xpool.tile([C_IN, IMG_PAD], _BF16, tag=f"x{n}",
                                name=f"x{n}")
                x_sb.append(xt)

            def xdma(eng, n, r0, r1):
                lo, hi = r0 * WP, r1 * WP
                eng.dma_start(x_sb[n][:, lo:hi], xp[n][:, lo:hi])

            nc.sync.dma_start(w_sb[:, 0:640], wt[:, 0:640])        # taps 0-4
            xdma(nc.scalar, 0, 0, 10)
            nc.sync.dma_start(w_sb[:, 640:1152], wt[:, 640:1152])  # taps 5-8
            xdma(nc.scalar, 0, 10, 22)
            nc.sync.dma_start(b_sb[:], bt[:])
            xdma(nc.sync, 0, 22, 34)
            xdma(nc.scalar, 0, 34, 46)
            xdma(nc.sync, 0, 46, 58)
            nc.scalar.dma_start(w_sb[:, W_COLS:], wt[:, W_COLS:])  # co-half 1
            for n in range(1, B_LOC):
                for r0, r1 in XN_ROW_CHUNKS:
                    xdma(nc.sync, n, r0, r1)

            def do_group(n, t, r0, nrows, store_eng, opool=opool,
                         act_on_dve=False):
                """One PSUM accumulation group: rows [r0, r0+nrows) of the
                output for image n, co-half t."""
                nmov = nrows * W_
                xv = x_sb[n][:].rearrange("p (h w) -> p h w", h=HP, w=WP)
                ps = ppool.tile([128, nmov], _F32, tag="ps")
                for k in range(N_TAPS):
                    kh, kw = divmod(k, 3)
                    rhs = xv[:, r0 + kh:r0 + kh + nrows, kw:kw + W_]
                    c0 = t * W_COLS + k * 128
                    nc.tensor.matmul(
                        ps[:], w_sb[:, c0:c0 + 128], rhs,
                        start=(k == 0),
                        stop=(k == N_TAPS - 1),
                    )
                o_sb = opool.tile([128, nmov], _F32, tag="o")
                if act_on_dve:
                    # Idle DVE does the PSUM->SBUF bias-add so the closing
                    # chain never waits on the Activation engine/SEQ.
                    nc.vector.tensor_scalar_add(o_sb[:], ps[:],
                                                b_sb[:, t:t + 1])
                else:
                    nc.scalar.activation(
                        o_sb[:], ps[:],
                        mybir.ActivationFunctionType.Identity,
                        bias=b_sb[:, t:t + 1],
                    )
                store_eng.dma_start(
                    out[n, bass.ts(t, 128), r0:r0 + nrows, :],
                    o_sb[:],
                )

            for n in range(B_LOC):
                for t in range(CO_TILES):
                    first_nt = (n == 0 and t == 0)
                    last_nt = (n == B_LOC - 1 and t == CO_TILES - 1)
                    if first_nt:
                        # The first 1-2 matmul instructions are costed at the
                        # mid p-state (they are dispatched before the warmup
                        # ramp completes) -- make them tiny so that penalty
                        # is small.
                        do_group(n, t, 0, 4, nc.scalar, opool=otail)
                        do_group(n, t, 4, 4, nc.scalar, opool=otail)
                        for j in range(1, N_BLK):
                            do_group(n, t, j * ROWS_PER_BLK, ROWS_PER_BLK,
                                     nc.scalar)
                    elif not last_nt:
                        for j in range(N_BLK):
                            do_group(n, t, j * ROWS_PER_BLK, ROWS_PER_BLK,
                                     nc.scalar)
                    else:
                        # Final (image, co-half): taper the block sizes so
                        # every act/store chain clears before the closer --
                        # j0..j4 full-size, two 6-row groups, then a 4-row
                        # closer whose bias-copy runs on the idle DVE and
                        # whose store goes out on the idle SP ring.
                        for j in range(5):
                            do_group(n, t, j * ROWS_PER_BLK, ROWS_PER_BLK,
                                     nc.scalar)
                        do_group(n, t, 40, 6, nc.sync, opool=otail)
                        do_group(n, t, 46, 6, nc.scalar, opool=otail)
                        # Closing 4 rows as two 2-row PSUM groups copied by
                        # DVE into one SBUF tile, flushed by a single store:
                        # the final copy is shorter, so the closing chain
                        # starts sooner.
                        o4 = otail.tile([128, 224], _F32, tag="o4")
                        xv = x_sb[n][:].rearrange("p (h w) -> p h w",
                                                  h=HP, w=WP)
                        for half, r0 in enumerate((52, 54)):
                            ps = ppool.tile([128, 112], _F32, tag="ps")
                            for k in range(N_TAPS):
                                kh, kw = divmod(k, 3)
                                rhs = xv[:, r0 + kh:r0 + kh + 2, kw:kw + W_]
                                c0 = t * W_COLS + k * 128
                                nc.tensor.matmul(
                                    ps[:], w_sb[:, c0:c0 + 128], rhs,
                                    start=(k == 0), stop=(k == N_TAPS - 1),
                                )
                            nc.vector.tensor_scalar_add(
                                o4[:, half * 112:(half + 1) * 112], ps[:],
                                b_sb[:, t:t + 1])
                        nc.sync.dma_start(
                            out[n, bass.ts(t, 128), 52:56, :], o4[:])

    _split_multi_waits(nc)
    return nc


_CACHED_NC = None


def _get_program():
    global _CACHED_NC
    if _CACHED_NC is None:
        _CACHED_NC = _build_program()
    return _CACHED_NC


def _prep_inputs(x, W, b):
    xp_all = np.pad(x, ((0, 0), (0, 0), (1, 1), (1, 1))).astype(
        ml_dtypes.bfloat16
    )
    # [C_out, C_in*9] -> [ci, (co_tile, tap, co)]
    wt = np.ascontiguousarray(
        W.reshape(CO_TILES, 128, C_IN, N_TAPS)
        .transpose(2, 0, 3, 1)
        .reshape(C_IN, -1)
    ).astype(ml_dtypes.bfloat16)
    bt = np.ascontiguousarray(b.reshape(CO_TILES, 128).T)
    in_maps = []
    for i in range(N_CORES):
        shard = np.ascontiguousarray(
            xp_all[i * B_LOC:(i + 1) * B_LOC].reshape(B_LOC, C_IN, IMG_PAD)
        )
        in_maps.append({"xp": shard, "wt": wt, "bt": bt})
    return in_maps


def kernel(x, W, b):
    x = np.asarray(x, dtype=np.float32)
    W = np.asarray(W, dtype=np.float32)
    b = np.asarray(b, dtype=np.float32)
    nc = _get_program()
    in_maps = _prep_inputs(x, W, b)
    res = run_bass_kernel_spmd(nc, in_maps, list(range(N_CORES)), trace=False)
    return np.concatenate([res.results[i]["out"] for i in range(N_CORES)], axis=0)


# revision 33
# speedup vs baseline: 1.0032x; 1.0009x over previous
"""Data-parallel 3x3 conv (implicit GEMM) for Trainium2, 8 NeuronCores.

Problem: x (32,128,56,56) f32, W (256,1152) f32 [C_out, C_in*KH*KW, taps in
(ci,kh,kw) order], b (256,), stride 1, pad 1 -> out (32,256,56,56) f32.

Strategy (cost-model span ~103.0us/core vs 94.25us pure-matmul floor)
- Shard the batch dim across the 8 cores (4 images each); replicate W and b.
- Host-side prep: zero-pad x to 58x58 and cast x and W to bf16. The PE runs
  bf16 at the same 1 cycle/row as f32r (full rate), input rounding adds
  ~2e-3 rel err (gate is 2e-2), input DMA bytes halve, and unlike f32r
  there is no N>=256 moving-dim constraint, which the small head/tail
  groups below rely on. W is pre-transposed to the stationary
  [ci, (co_tile, tap, co)] layout so the first co-half streams in first.
- All loads are plain HWDGE DMAs (no SWDGE descriptor generation, no
  casting), interleaved over the two HWDGE rings (sync/SP and scalar/Act)
  so ring prep pipelines; chunk order is tuned so the first W taps and the
  first x rows land just as the PE p-state warm-up completes (~4.4us) and
  every later chunk arrives before its first consumer group.
- Per core: keep the whole padded bf16 shard (3.4 MB) + weights in SBUF.
  For each (image, co-half, row block): accumulate 9 shifted matmuls (one
  per 3x3 tap) into one PSUM bank, contraction dim = C_in = 128, moving
  dim N = rows*56 (448 for the 8-row steady-state blocks). ScalarE fuses
  the bias-add with the PSUM->SBUF copy and stores from its HWDGE ring.
  The matmul stream is gap-free: 504 tap-matmuls back-to-back.
- Head/tail shaping: the first block is split 4+4 so the two matmuls that
  get costed at the mid p-state are small; the final block tapers
  6+6+2+2, where the two closing 2-row groups are bias-copied by the
  otherwise idle DVE into one shared SBUF tile flushed by a single store
  on the idle SP ring, so the closing copy+store+sem+drain chain after
  the last matmul is ~3.9us instead of ~4.7us.
"""

import numpy as np
import ml_dtypes

import bass_rust as _br
import concourse.bass as bass
import concourse.mybir as mybir
import concourse.tile as tile
from concourse.bass_utils import run_bass_kernel_spmd

N_CORES = 8
B, C_IN, H, W_ = 32, 128, 56, 56
C_OUT = 256
B_LOC = B // N_CORES          # 4 images per core
HP, WP = H + 2, W_ + 2        # padded 58x58
IMG_PAD = HP * WP             # 3364
ROWS_PER_BLK = 8              # 8 rows * 56 cols = 448 = moving dim
N_BLK = H // ROWS_PER_BLK     # 7
N_MOV = ROWS_PER_BLK * W_     # 448
N_TAPS = 9
CO_TILES = C_OUT // 128       # 2

_F32 = mybir.dt.float32
_BF16 = mybir.dt.bfloat16

# x image-0 row chunks (DMA'd individually so the PE can start early);
# remaining images load in halves.
X0_ROW_CHUNKS = [(0, 10), (10, 22), (22, 34), (34, 46), (46, 58)]
XN_ROW_CHUNKS = [(0, 29), (29, 58)]
# W columns in [ci, (co_tile, tap, co)] layout: first co-half split so the
# first tap group streams just ahead of the matmuls.
W_COLS = N_TAPS * 128         # 1152 per co-half
W_CHUNKS = [(0, 640), (640, 1152)]   # co-half 0 (taps 0-4, taps 5-8)
N_WARMUP = 28                 # small dummy matmuls to ramp the PE clock


def _split_multi_waits(nc):
    """This walrus build accepts at most ONE sync-wait per instruction.

    Tile can emit several (e.g. a matmul waiting on two input DMAs, or the
    exit drain waiting on every outstanding semaphore). Hoist the extras onto
    injected same-engine NOPs immediately ahead of the offender — sequencers
    execute their stream in order, so the waits still all happen before it.
    """
    for bb in nc.m.functions[0].blocks:
        il = bb.instructions
        i = 0
        while i < len(il):
            inst = il[i]
            si = inst.sync_info
            w = list(si.on_wait) if (si and si.on_wait) else []
            if len(w) > 1:
                si.on_wait = w[-1:]
                for wi in w[:-1]:
                    nop = mybir.InstNoOp(
                        name=nc.get_next_instruction_name(), ins=[], outs=[]
                    )
                    nop.engine = inst.engine
                    nop.sync_info = _br.SyncInfo(on_wait=[wi], on_update=[])
                    nc.register_instruction(nop)
                    il.insert(i, nop)
                    i += 1
            i += 1


def _build_program():
    nc = bass.Bass("TRN2", target_bir_lowering=False, debug=False,
                   num_devices=N_CORES)
    xp = nc.dram_tensor("xp", [B_LOC, C_IN, IMG_PAD], _BF16,
                        kind="ExternalInput").ap()
    wt = nc.dram_tensor("wt", [C_IN, CO_TILES * W_COLS], _BF16,
                        kind="ExternalInput").ap()
    bt = nc.dram_tensor("bt", [128, CO_TILES], _F32, kind="ExternalInput").ap()
    out = nc.dram_tensor("out", [B_LOC, C_OUT, H, W_], _F32,
                         kind="ExternalOutput").ap()

    with tile.TileContext(nc) as tc:
        with (
            tc.tile_pool(name="xpool", bufs=1) as xpool,
            tc.tile_pool(name="wpool", bufs=1) as wpool,
            tc.tile_pool(name="opool", bufs=4) as opool,
            tc.tile_pool(name="otail", bufs=3) as otail,
            tc.tile_pool(name="wmpool", bufs=1, space="PSUM") as wmpool,
            tc.tile_pool(name="ppool", bufs=4, space="PSUM") as ppool,
        ):
            # PE warm-up: the HAM clock-gate runs the PE at reduced speed
            # until it sees ~3us of sustained activity. The real stream can't
            # start until the first weight/activation chunks land (~3.4us),
            # so burn that window on dummy matmuls over a zeroed scratch tile
            # into a never-read PSUM bank.
            scratch = wpool.tile([128, 128], _BF16, tag="scratch")
            nc.gpsimd.memset(scratch[:], 0.0)
            wps = wmpool.tile([128, 128], _F32, tag="wps")
            for _ in range(N_WARMUP):
                nc.tensor.matmul(wps[:], scratch[:], scratch[:],
                                 start=True, stop=True)

            # Only SP (nc.sync) and Activation (nc.scalar) have HWDGE rings.
            # Startup-critical chunks alternate between them so ring prep
            # overlaps; the bulk image loads go on SP (Act issues the 57
            # activation+store pairs later).
            w_sb = wpool.tile([C_IN, CO_TILES * W_COLS], _BF16, tag="w")
            b_sb = wpool.tile([128, CO_TILES], _F32, tag="b")
            x_sb = []
            for n in range(B_LOC):
                xt = xpool.tile([C_IN, IMG_PAD], _BF16, tag=f"x{n}",
                                name=f"x{n}")
                x_sb.append(xt)

            def xdma(eng, n, r0, r1):
                lo, hi = r0 * WP, r1 * WP
                eng.dma_start(x_sb[n][:, lo:hi], xp[n][:, lo:hi])

            nc.sync.dma_start(w_sb[:, 0:640], wt[:, 0:640])        # taps 0-4
            xdma(nc.scalar, 0, 0, 10)
            nc.sync.dma_start(w_sb[:, 640:1152], wt[:, 640:1152])  # taps 5-8
            xdma(nc.scalar, 0, 10, 22)
            nc.sync.dma_start(b_sb[:], bt[:])
            xdma(nc.sync, 0, 22, 34)
            xdma(nc.scalar, 0, 34, 46)
            xdma(nc.sync, 0, 46, 58)
            nc.scalar.dma_start(w_sb[:, W_COLS:], wt[:, W_COLS:])  # co-half 1
            for n in range(1, B_LOC):
                for r0, r1 in XN_ROW_CHUNKS:
                    xdma(nc.sync, n, r0, r1)

            def do_group(n, t, r0, nrows, store_eng, opool=opool,
                         act_on_dve=False):
                """One PSUM accumulation group: rows [r0, r0+nrows) of the
                output for image n, co-half t."""
                nmov = nrows * W_
                xv = x_sb[n][:].rearrange("p (h w) -> p h w", h=HP, w=WP)
                ps = ppool.tile([128, nmov], _F32, tag="ps")
                for k in range(N_TAPS):
                    kh, kw = divmod(k, 3)
                    rhs = xv[:, r0 + kh:r0 + kh + nrows, kw:kw + W_]
                    c0 = t * W_COLS + k * 128
                    nc.tensor.matmul(
                        ps[:], w_sb[:, c0:c0 + 128], rhs,
                        start=(k == 0),
                        stop=(k == N_TAPS - 1),
                    )
                o_sb = opool.tile([128, nmov], _F32, tag="o")
                if act_on_dve:
                    # Idle DVE does the PSUM->SBUF bias-add so the closing
                    # chain never waits on the Activation engine/SEQ.
                    nc.vector.tensor_scalar_add(o_sb[:], ps[:],
                                                b_sb[:, t:t + 1])
                else:
                    nc.scalar.activation(
                        o_sb[:], ps[:],
                        mybir.ActivationFunctionType.Identity,
                        bias=b_sb[:, t:t + 1],
                    )
                store_eng.dma_start(
                    out[n, bass.ts(t, 128), r0:r0 + nrows, :],
                    o_sb[:],
                )

            for n in range(B_LOC):
                for t in range(CO_TILES):
                    first_nt = (n == 0 and t == 0)
                    last_nt = (n == B_LOC - 1 and t == CO_TILES - 1)
                    if first_nt:
                        # The first 1-2 matmul instructions are costed at the
                        # mid p-state (they are dispatched before the warmup
                        # ramp completes) -- make them tiny so that penalty
                        # is small: tap 0 of the very first group is emitted
                        # as two column-disjoint 2-row matmuls (both
                        # start=True over their own PSUM region).
                        xv0 = x_sb[0][:].rearrange("p (h w) -> p h w",
                                                   h=HP, w=WP)
                        ps0 = ppool.tile([128, 4 * W_], _F32, tag="ps")
                        for half in range(2):
                            rhs = xv0[:, 2 * half:2 * half + 2, 0:W_]
                            nc.tensor.matmul(
                                ps0[:, half * 112:(half + 1) * 112],
                                w_sb[:, 0:128], rhs,
                                start=True, stop=False,
                                skip_group_check=True,
                            )
                        for k in range(1, N_TAPS):
                            kh, kw = divmod(k, 3)
                            rhs = xv0[:, kh:kh + 4, kw:kw + W_]
                            nc.tensor.matmul(
                                ps0[:], w_sb[:, k * 128:k * 128 + 128], rhs,
                                start=False, stop=(k == N_TAPS - 1),
                                skip_group_check=True,
                            )
                        o0 = otail.tile([128, 4 * W_], _F32, tag="o")
                        nc.scalar.activation(
                            o0[:], ps0[:],
                            mybir.ActivationFunctionType.Identity,
                            bias=b_sb[:, t:t + 1],
                        )
                        nc.scalar.dma_start(out[0, bass.ts(0, 128), 0:4, :],
                                            o0[:])
                        do_group(n, t, 4, 4, nc.scalar, opool=otail)
                        for j in range(1, N_BLK):
                            do_group(n, t, j * ROWS_PER_BLK, ROWS_PER_BLK,
                                     nc.scalar)
                    elif not last_nt:
                        for j in range(N_BLK):
                            do_group(n, t, j * ROWS_PER_BLK, ROWS_PER_BLK,
                                     nc.scalar)
                    else:
                        # Final (image, co-half): taper the block sizes so
                        # every act/store chain clears before the closer --
                        # j0..j4 full-size, two 6-row groups, then a 4-row
                        # closer whose bias-copy runs on the idle DVE and
                        # whose store goes out on the idle SP ring.
                        for j in range(5):
                            do_group(n, t, j * ROWS_PER_BLK, ROWS_PER_BLK,
                                     nc.scalar)
                        do_group(n, t, 40, 6, nc.sync, opool=otail)
                        do_group(n, t, 46, 6, nc.scalar, opool=otail)
                        # Closing 4 rows as two 2-row PSUM groups copied by
                        # DVE into one SBUF tile, flushed by a single store:
                        # the final copy is shorter, so the closing chain
                        # starts sooner.
                        o4 = otail.tile([128, 224], _F32, tag="o4")
                        xv = x_sb[n][:].rearrange("p (h w) -> p h w",
                                                  h=HP, w=WP)
                        for half, r0 in enumerate((52, 54)):
                            ps = ppool.tile([128, 112], _F32, tag="ps")
                            for k in range(N_TAPS):
                                kh, kw = divmod(k, 3)
                                rhs = xv[:, r0 + kh:r0 + kh + 2, kw:kw + W_]
                                c0 = t * W_COLS + k * 128
                                nc.tensor.matmul(
                                    ps[:], w_sb[:, c0:c0 + 128], rhs,
                                    start=(k == 0), stop=(k == N_TAPS - 1),
                                )
                            nc.vector.tensor_scalar_add(
                                o4[:, half * 112:(half + 1) * 112], ps[:],
                                b_sb[:, t:t + 1])
                        nc.sync.dma_start(
                            out[n, bass.ts(t, 128), 52:56, :], o4[:])

    _split_multi_waits(nc)
    return nc


_CACHED_NC = None


def _get_program():
    global _CACHED_NC
    if _CACHED_NC is None:
        _CACHED_NC = _build_program()
    return _CACHED_NC


def _prep_inputs(x, W, b):
    xp_all = np.pad(x, ((0, 0), (0, 0), (1, 1), (1, 1))).astype(
        ml_dtypes.bfloat16
    )
    # [C_out, C_in*9] -> [ci, (co_tile, tap, co)]
    wt = np.ascontiguousarray(
        W.reshape(CO_TILES, 128, C_IN, N_TAPS)
        .transpose(2, 0, 3, 1)
        .reshape(C_IN, -1)
    ).astype(ml_dtypes.bfloat16)
    bt = np.ascontiguousarray(b.reshape(CO_TILES, 128).T)
    in_maps = []
    for i in range(N_CORES):
        shard = np.ascontiguousarray(
            xp_all[i * B_LOC:(i + 1) * B_LOC].reshape(B_LOC, C_IN, IMG_PAD)
        )
        in_maps.append({"xp": shard, "wt": wt, "bt": bt})
    return in_maps


def kernel(x, W, b):
    x = np.asarray(x, dtype=np.float32)
    W = np.asarray(W, dtype=np.float32)
    b = np.asarray(b, dtype=np.float32)
    nc = _get_program()
    in_maps = _prep_inputs(x, W, b)
    res = run_bass_kernel_spmd(nc, in_maps, list(range(N_CORES)), trace=False)
    return np.concatenate([res.results[i]["out"] for i in range(N_CORES)], axis=0)
